# revision 60
# baseline (speedup 1.0000x reference)
"""Causal multi-head attention on 8 Trainium2 NeuronCores.

Problem: x[B=4,S=2048,E=1024], Wq/Wk/Wv[H=16,E,D=64], Wo[E,E], bo[E].
  out = softmax_causal(q k^T / sqrt(D)) v, heads concat, @ Wo.T + bo

Sharding (tensor parallel over heads, data parallel over batch):
  core c -> (batch b = c//2, head-group g = c%2 of 8 heads).
  Each core: QKV projections + attention for its 8 heads of its batch.
  Output exchange: instead of ReduceScatter-ing full-width output
  partials (v3), each core AllGathers its NORMALIZED attention tiles
  (one [128, 512] block per head-pair d) with its pair peer, then
  computes the output projection for ONLY its own 512 output columns
  with the full 1024-dim contraction (Wo columns sliced per core, full
  rows).  Same PE work, no partial-sum adds, collective payloads are
  small and pipeline with compute.  Collectives cost ~10-20us each
  regardless of size (latency-bound), so there are exactly 5: one per
  chunk (popped as fills one chunk later, a full chunk before that
  chunk's projection needs the data), and the last chunk splits into an
  early d0+d1 exchange (fires ~50% through the chunk, always done
  before the tail) and a final one carrying d2-normalized + d3-RAW +
  d3's two denominator rows.  Shipping d3 raw moves the whole
  reciprocal/broadcast/normalize chain off the pre-AllGather critical
  path: both sides normalize the gathered d3 blocks identically while
  the projection's first 16 matmuls (kd 0,1,4,5, resident since the
  early exchange) and a string of dead warm-keeper matmuls (HAM stays
  at 2.4 GHz) bridge the AllGather latency; kd 2,6 join at readback and
  kd 3,7 land last.

Kernel internals (per core):
  - All SBUF data bf16 (psum f32).  Activations transposed: xT[E,S],
    QT/KT[dg,S], scoresT[t,q]; softmax denominator from a ones-column
    appended to V; probabilities feed the AV matmul moving operand.
  - Scores for a head PAIR (row-tiled 64-contraction matmuls at PE row
    groups 0/64) share one [128, 1024] psum tile; ONE exp activation
    covers both heads.
  - Causality at 128 granularity: suffix-restricted scores/exp/AV on
    diagonal tiles + a [128,128] upper-tri wedge multiply.
  - Denominator rows are extracted right after each AV accumulation
    (vector queue only, never stalls PE); each chunk's reciprocal/
    broadcast/normalize runs as fills in the NEXT chunk, with the dinv
    fill placed several iterations before the first norm so the PE
    broadcast matmul never waits on the reciprocal chain.
  - Phase-1 (QKV) and out-projection work interleaved into the attention
    emission so TensorE always has dense work while ScalarE exp runs;
    late K/V emission is shifted into the ACT-bound last chunk.
"""

import os
import sys

for _p in ("/opt/trn_rl_repo", "/root/.axon_site/_ro/trn_rl_repo"):
    if os.path.isdir(_p) and _p not in sys.path:
        sys.path.append(_p)

import numpy as np
import ml_dtypes

import concourse.bass as bass
import concourse.mybir as mybir
import concourse.tile as tile
from concourse import bacc

B, S, E, H, D = 4, 2048, 1024, 16, 64
NCORES = 8
G = 2  # head groups
HL = H // G  # heads per core = 8
DG = HL * D  # local head dim = 512
EH = E // G  # final output columns per core = 512
KD = 2 * (DG // 128)  # global 128-dim blocks of the full head dim = 8
ETM = EH // 128  # my output col blocks = 4
P = 128
SC = 512  # sequence chunk
NSC = S // SC  # 4
NT = S // P  # 16 key tiles
ET = E // P  # 8 embedding tiles
ND = DG // P  # 4 head-pairs per core
SCALE = 1.0 / np.sqrt(D)
PAIRS = [[0, 1], [2, 3], [4, 5], [6, 7]]

F32 = mybir.dt.float32
BF16 = mybir.dt.bfloat16

_CACHE = {}


def _build_nc():
    nc = bacc.Bacc("TRN2", target_bir_lowering=False, debug=False, num_devices=NCORES)

    xT = nc.dram_tensor("xT", [E, S], BF16, kind="ExternalInput")
    wq = nc.dram_tensor("wq", [E, DG], BF16, kind="ExternalInput")
    wk = nc.dram_tensor("wk", [E, DG], BF16, kind="ExternalInput")
    wv = nc.dram_tensor("wv", [E, DG], BF16, kind="ExternalInput")
    wo2 = nc.dram_tensor("wo2", [2 * DG, EH], BF16, kind="ExternalInput")
    bo2 = nc.dram_tensor("bo2", [P, ETM], F32, kind="ExternalInput")
    mask = nc.dram_tensor("mask", [P, P], BF16, kind="ExternalInput")
    sel8 = nc.dram_tensor("sel8", [HL, ND * P], BF16, kind="ExternalInput")
    sel2 = nc.dram_tensor("sel2", [2, P], BF16, kind="ExternalInput")
    selh = nc.dram_tensor("selh", [1, 2 * P], BF16, kind="ExternalInput")
    outT = nc.dram_tensor("outT", [NSC, EH, SC], BF16, kind="ExternalOutput")

    with tile.TileContext(nc) as tc:
        with (
            tc.tile_pool(name="persist", bufs=1) as persist,
            tc.tile_pool(name="expp", bufs=5) as expp,
            tc.tile_pool(name="attnp", bufs=8) as attnp,
            tc.tile_pool(name="agp", bufs=16) as agp,
            tc.tile_pool(name="dnp", bufs=4) as dnp,
            tc.tile_pool(name="dinvp", bufs=3) as dinvp,
            tc.tile_pool(name="workp", bufs=4) as workp,
            tc.tile_pool(name="outp", bufs=4) as outp,
            tc.tile_pool(name="psc", bufs=2, space="PSUM") as psc,
            tc.tile_pool(name="patt", bufs=2, space="PSUM") as patt,
            tc.tile_pool(name="pmisc", bufs=2, space="PSUM") as pmisc,
            tc.tile_pool(name="dram", bufs=1, space="DRAM") as dram,
        ):
            # ---- persistent tiles ----
            xs = [
                [persist.tile([P, SC], BF16, name=f"x{e}_{c}") for c in range(NSC)]
                for e in range(ET)
            ]
            wq_sb = persist.tile([P, ET, DG], BF16, name="wq")
            wk_sb = persist.tile([P, ET, DG], BF16, name="wk")
            wv_sb = persist.tile([P, ET, DG], BF16, name="wv")
            wo_sb = persist.tile([P, KD, EH], BF16, name="wo")
            bo_sb = persist.tile([P, ETM], F32, name="bo")
            mask_sb = persist.tile([P, P], BF16, name="mask")
            sel8_sb = persist.tile([HL, ND * P], BF16, name="sel8")
            sel2_sb = persist.tile([2, P], BF16, name="sel2")
            selh_sb = persist.tile([1, 2 * P], BF16, name="selh")
            kt = [
                [persist.tile([P, SC], BF16, name=f"kt{d}_{kc}") for kc in range(NSC)]
                for d in range(ND)
            ]
            qt = [
                [persist.tile([P, SC], BF16, name=f"qt{d}_{sc}") for sc in range(NSC)]
                for d in range(ND)
            ]
            v_sb = [persist.tile([P, HL, D + 1], BF16, name=f"v{t}") for t in range(NT)]

            cc_at = dram.tile([NSC, ND, P, SC], BF16)
            cc_ag = [dram.tile([2, ND, P, SC], BF16, name=f"ccag{c}") for c in range(3)]
            cc_ag3a = dram.tile([2, 2, P, SC], BF16)
            # final exchange: d2 normalized + d3 RAW + 2 denominator rows
            cc_at3b = dram.tile([2 * P + 2, SC], BF16)
            cc_ag3b = dram.tile([2, 2 * P + 2, SC], BF16)
            # throwaway payload: a dummy AllGather right before the final one
            # absorbs the cold-start cost of the collective stream
            cc_warm = dram.tile([1, SC], BF16)
            cc_warmg = dram.tile([2, SC], BF16)

            # ---- input DMAs: per-e weight blocks interleaved with x slices
            # so K(0)'s e-th matmul can start as soon as its operands land.
            # The first-needed blocks go on the Scalar (ACT) hwdge queue,
            # whose engine preamble retires earlier than Sync's.
            # tiny constants on the otherwise-idle scalar queue: the wedge
            # mask gates every diagonal AV of chunk 0
            nc.scalar.dma_start(mask_sb[:], mask[:])
            nc.scalar.dma_start(sel8_sb[:], sel8[:])
            nc.scalar.dma_start(sel2_sb[:], sel2[:])
            nc.scalar.dma_start(selh_sb[:], selh[:])
            nc.scalar.dma_start(bo_sb[:], bo2[:])
            for e in range(ET):
                nc.sync.dma_start(
                    wk_sb[:, e, :], wk[P * e : P * (e + 1), :]
                )
                nc.sync.dma_start(xs[e][0][:], xT[P * e : P * (e + 1), 0:SC])
            for e in range(ET):
                nc.sync.dma_start(wq_sb[:, e, :], wq[P * e : P * (e + 1), :])
            for e in range(ET):
                nc.sync.dma_start(wv_sb[:, e, :], wv[P * e : P * (e + 1), :])
            for c in range(1, NSC):
                for e in range(ET):
                    nc.sync.dma_start(
                        xs[e][c][:], xT[P * e : P * (e + 1), SC * c : SC * (c + 1)]
                    )
            nc.sync.dma_start(wo_sb[:], wo2.rearrange("(ko p) m -> p ko m", p=P))
            for t in range(NT):
                nc.vector.memset(v_sb[t][:, :, D], 1.0)
            # zero the two score-psum ring slots once: merged diagonal exps
            # read a junk gap between the two halves, which must be finite
            for _ in range(2):
                sco0 = psc.tile([P, 2 * SC], F32, tag="sc", name="sco_init")
                nc.vector.memset(sco0[:], 0.0)

            # ---- phase-1 emitters ----
            def emit_k(d, kc, w_sb=wk_sb, dst=kt, sce=False):
                acc = pmisc.tile([P, SC], F32, tag="m", name="acc")
                for e in range(ET):
                    nc.tensor.matmul(
                        acc[:],
                        w_sb[:, e, P * d : P * (d + 1)],
                        xs[e][kc][:],
                        start=(e == 0),
                        stop=(e == ET - 1),
                    )
                # sce: free the psum slot via the (idle) scalar engine so the
                # next fill's matmul never waits on a backed-up DVE queue
                (nc.scalar.copy if sce else nc.vector.tensor_copy)(
                    dst[d][kc][:], acc[:]
                )

            def emit_q(d, sc, sce=False):
                emit_k(d, sc, w_sb=wq_sb, dst=qt, sce=sce)

            def emit_v(t, sce=False):
                acc = pmisc.tile([P, DG], F32, tag="m", name="accv")
                for e in range(ET):
                    nc.tensor.matmul(
                        acc[:],
                        xs[e][t // 4][:, P * (t % 4) : P * (t % 4 + 1)],
                        wv_sb[:, e, :],
                        start=(e == 0),
                        stop=(e == ET - 1),
                    )
                (nc.scalar.copy if sce else nc.vector.tensor_copy)(
                    v_sb[t][:, :, 0:D], acc[:].rearrange("p (h d) -> p h d", d=D)
                )

            # ---- per-chunk epilogue: normalize, AllGather, project ----
            # kd order is GLOBAL: rank0's d0..3 -> kd 0..3, rank1's -> 4..7,
            # matching wo2's row order (full head-dim, global head index).
            ag_tiles = [[None] * KD for _ in range(NSC)]
            at_tiles = [[None] * ND for _ in range(NSC)]
            dn_locs = [None] * NSC  # [HL, SC] f32, denominators of a chunk

            def op_dinv(dn_ap, rows, dinv_b, sce=False):
                dinv_f = dinvp.tile([rows, SC], F32, tag="dif", name="dinv_f")
                nc.vector.reciprocal_approx_fast(dinv_f[:], dn_ap)
                (nc.scalar.copy if sce else nc.vector.tensor_copy)(
                    dinv_b[:], dinv_f[:]
                )

            def op_norm(sc, d, sel_ap, dinv_b, sce=False):
                # broadcast 1/den across each head's 64 rows, normalize the
                # raw AV tile in place, stage it for the AllGather
                bc_ps = pmisc.tile([P, SC], F32, tag="m", name="bc_ps")
                nc.tensor.matmul(bc_ps[:], sel_ap, dinv_b[:], start=True, stop=True)
                bc = workp.tile([P, SC], BF16, tag="bc")
                (nc.scalar.copy if sce else nc.vector.tensor_copy)(bc[:], bc_ps[:])
                at_d = at_tiles[sc][d]
                nc.vector.tensor_mul(at_d[:], at_d[:], bc[:])
                nc.sync.dma_start(cc_at[sc, d], at_d[:])

            def op_ag(in_ap, out_ap):
                nc.gpsimd.collective_compute(
                    "AllGather",
                    mybir.AluOpType.bypass,
                    replica_groups=PAIRS,
                    ins=[in_ap.opt()],
                    outs=[out_ap.opt()],
                )

            def op_readback(sc, out_tile, ds):
                # pull both ranks' gathered blocks back into SBUF
                for r in range(2):
                    for j, d in enumerate(ds):
                        a = agp.tile([P, SC], BF16, tag="ag", name=f"ag{sc}_{r}_{d}")
                        nc.sync.dma_start(a[:], out_tile[r, j])
                        ag_tiles[sc][ND * r + d] = a

            # last-chunk accumulation order: the blocks fed by the final
            # AllGather (kd 3 and 7) go last so the rest overlaps it
            KD_ORDER = [0, 1, 2, 4, 5, 6, 3, 7]

            def op_proj(sc, et):
                acc = pmisc.tile([P, SC], F32, tag="m", name="acco")
                for i, kd in enumerate(KD_ORDER):
                    nc.tensor.matmul(
                        acc[:],
                        wo_sb[:, kd, P * et : P * (et + 1)],
                        ag_tiles[sc][kd][:],
                        start=(i == 0),
                        stop=(i == KD - 1),
                    )
                stage = outp.tile([P, SC], BF16, tag="ot")
                nc.vector.tensor_scalar_add(stage[:], acc[:], bo_sb[:, et : et + 1])
                nc.sync.dma_start(outT[sc, P * et : P * (et + 1), :], stage[:])

            def epi_pieces(sc):
                # epilogue pieces for chunks 0..2, interleaved as fills in
                # chunk sc+1: (dinv, [norm x4], ag+readback)
                dinv_b = dinvp.tile([HL, SC], BF16, tag="dib", name="dinv_b")
                f_dinv = lambda: op_dinv(dn_locs[sc][:], HL, dinv_b)
                f_norms = [
                    (
                        lambda d=d: op_norm(
                            sc, d, sel8_sb[:, P * d : P * (d + 1)], dinv_b
                        )
                    )
                    for d in range(ND)
                ]

                def f_agrb():
                    op_ag(cc_at[sc], cc_ag[sc])
                    op_readback(sc, cc_ag[sc], range(ND))

                return f_dinv, f_norms, f_agrb

            # ---- attention ----
            # `urgent` holds the last chunk's own mid-chunk epilogue
            # closures: each entry is (ready_git, fn) and pops only a few
            # iterations after creation so its PE matmul never waits on the
            # freshly-issued reciprocal chain.
            urgent = []
            git = [0]  # global iteration counter across chunks
            last = NSC - 1
            dn2p = [None]  # last chunk d2's denominators, partition-0 based

            def attention_chunk(sc, fills):
                nt = 4 * (sc + 1)
                n_iter = ND * nt
                it = 0
                popped = 0
                dn_locs[sc] = dnp.tile([HL, SC], F32, tag="dnl", name=f"dn{sc}")

                for d in range(ND):
                    att = [
                        patt.tile([D + 1, SC], F32, tag="att", name=f"att{h}")
                        for h in range(2)
                    ]

                    def emit_av(j, o, ex, d=d, att=att, nt=nt):
                        for half in range(2):
                            nc.tensor.matmul(
                                att[half][:, o:SC],
                                v_sb[j][:, 2 * d + half, :],
                                ex[:, SC * half + o : SC * (half + 1)],
                                start=(j == 0),
                                stop=(j == nt - 1),
                            )

                    prev = None
                    for j in range(nt):
                        m = j - 4 * sc
                        o = P * m if m > 0 else 0
                        sco = psc.tile([P, 2 * SC], F32, tag="sc", name="sco")
                        for half in range(2):
                            r = D * half
                            nc.tensor.matmul(
                                sco[:, SC * half + o : SC * (half + 1)],
                                kt[d][j // 4][r : r + D, P * (j % 4) : P * (j % 4 + 1)],
                                qt[d][sc][r : r + D, o:SC],
                                start=True,
                                stop=True,
                                tile_position=(r, 0),
                            )
                        ex = expp.tile([P, 2 * SC], BF16, tag="ex", name="ex")
                        # one exp covers both halves; for diagonal tiles it
                        # spans the unwritten gap between them (finite junk,
                        # never read downstream) to save the per-op overhead
                        nc.scalar.activation(
                            ex[:, o : 2 * SC],
                            sco[:, o : 2 * SC],
                            mybir.ActivationFunctionType.Exp,
                            scale=SCALE,
                        )
                        if m >= 0:  # diagonal tile: mask the wedge
                            for half in range(2):
                                c0 = SC * half + P * m
                                nc.vector.tensor_mul(
                                    ex[:, c0 : c0 + P], ex[:, c0 : c0 + P], mask_sb[:]
                                )
                        if prev is not None:
                            emit_av(*prev)
                        prev = (j, o, ex)
                        it += 1
                        git[0] += 1
                        if urgent and git[0] >= urgent[0][0]:
                            urgent.pop(0)[1]()
                        want = (it * len(fills)) // n_iter + 1
                        while popped < min(want, len(fills)):
                            fills[popped]()
                            popped += 1
                    emit_av(*prev)
                    # stage raw AV + extract denominators (vector/DMA only;
                    # nothing here blocks the PE queue)
                    attn_t = attnp.tile([P, SC], BF16, tag="at", name="attn_t")
                    at_tiles[sc][d] = attn_t
                    if sc == last and d == ND - 1:
                        # final head-pair: ship the UNNORMALIZED AV plus its
                        # two denominator rows (packed on partition 0); both
                        # sides normalize after the exchange, off the pre-AG
                        # critical path
                        dn3p = dnp.tile([1, 2 * SC], F32, tag="dn3", name="dn3p", bufs=1)
                        for half in range(2):
                            nc.scalar.copy(
                                dn3p[0:1, SC * half : SC * (half + 1)],
                                att[half][D : D + 1, :],
                            )
                            nc.vector.tensor_copy(
                                attn_t[D * half : D * (half + 1), :],
                                att[half][0:D, :],
                            )
                        # reciprocal BEFORE the exchange: both sides receive
                        # ready-to-use 1/den rows, so the post-AG chain is
                        # just broadcast-multiply-project
                        dinv_f3 = dinvp.tile(
                            [1, 2 * SC], F32, tag="dif3", name="dinv_f3", bufs=1
                        )
                        nc.vector.reciprocal_approx_fast(dinv_f3[:], dn3p[0:1, :])
                        dinv3b = dnp.tile(
                            [1, 2 * SC], BF16, tag="dn3b", name="dinv3b", bufs=1
                        )
                        nc.scalar.copy(dinv3b[:], dinv_f3[:])
                        nc.scalar.dma_start(cc_at3b[P : 2 * P, :], attn_t[:])
                        nc.scalar.dma_start(
                            cc_at3b[2 * P : 2 * P + 2, :], dinv3b[0:1, :]
                        )
                        op_ag(cc_at3b, cc_ag3b)
                        continue
                    cp = nc.scalar.copy if sc == 0 else nc.vector.tensor_copy
                    for half in range(2):
                        cp(
                            attn_t[D * half : D * (half + 1), :], att[half][0:D, :]
                        )
                        dnrow = dnp.tile([1, SC], F32, tag="dn", name="dnrow", bufs=2)
                        cp(dnrow[:], att[half][D : D + 1, :])
                        if sc == last and d == 2:
                            if dn2p[0] is None:
                                dn2p[0] = dnp.tile(
                                    [2, SC], F32, tag="dn2p", name="dn2p", bufs=1
                                )
                            nc.sync.dma_start(
                                dn2p[0][half : half + 1, :], dnrow[:]
                            )
                        else:
                            row = 2 * d + half
                            nc.sync.dma_start(
                                dn_locs[sc][row : row + 1, :], dnrow[:]
                            )
                    if sc == last and d == 1:
                        # the last chunk's first two head-pairs normalize and
                        # AllGather mid-chunk (always done before the tail);
                        # delayed pops keep PE off the recip chain
                        dinv4 = dinvp.tile([4, SC], BF16, tag="dib", name="dinv4")
                        urgent.extend(
                            [
                                (
                                    git[0] + 3,
                                    lambda: op_dinv(dn_locs[sc][0:4, :], 4, dinv4),
                                ),
                            ]
                            + [
                                (
                                    git[0] + 4 + dd,
                                    lambda dd=dd: op_norm(
                                        sc,
                                        dd,
                                        sel8_sb[0:4, P * dd : P * (dd + 1)],
                                        dinv4,
                                    ),
                                )
                                for dd in range(2)
                            ]
                            + [
                                (
                                    git[0] + 9,
                                    lambda: op_ag(cc_at[sc, 0:2], cc_ag3a),
                                ),
                                (
                                    git[0] + 10,
                                    lambda: op_readback(sc, cc_ag3a, range(2)),
                                ),
                            ]
                        )
                    if sc == last and d == 2:
                        # d2 normalizes mid-chunk and stages into the final
                        # exchange buffer (its dn rows live on partitions 0-1)
                        dinv2p = dinvp.tile([2, SC], BF16, tag="dib", name="dinv2p")
                        urgent.extend(
                            [
                                (git[0] + 3, lambda: op_dinv(dn2p[0][:], 2, dinv2p)),
                                (
                                    git[0] + 4,
                                    lambda: op_norm(sc, 2, sel2_sb[:], dinv2p),
                                ),
                                (
                                    git[0] + 5,
                                    lambda: nc.sync.dma_start(
                                        cc_at3b[0:P, :], at_tiles[sc][2][:]
                                    ),
                                ),
                                # the dummy AG must not be hoisted: feed it
                                # from a tile that only exists late so it
                                # runs right before the final exchange
                                (
                                    git[0] + 6,
                                    lambda: nc.sync.dma_start(
                                        cc_warm[0:1, :], dinv2p[0:1, :]
                                    ),
                                ),
                                (git[0] + 8, lambda: op_ag(cc_warm, cc_warmg)),
                            ]
                        )
                while popped < len(fills):
                    fills[popped]()
                    popped += 1

            # ---- schedule ----
            # minimal prologue: only what chunk-0 d0's attention needs
            emit_k(0, 0, sce=True)
            emit_q(0, 0, sce=True)
            for t in range(4):
                emit_v(t, sce=True)

            # fill layout per chunk (positions paced over the chunk, +1
            # prefetch).  Emitters sit in the LAST chunk that still meets
            # their first-use iteration, pushing PE work into the exp-bound
            # tail chunks:
            #   chunk0: [K/Q d1-3, Q(1)x4, K(1)x4, V4-7]
            #   chunk1: [Q(2)x4, dinv(0), K(2)x4, norm(0)x4, V10, V11, agrb0]
            #   chunk2: [V8, V9, Q(3)x4, dinv(1), norm(1)x4, agrb1, proj(0)x4]
            #   chunk3: [K30, V14, K31, V15, K32, K33, dinv(2), norm(2)x4,
            #            agrb2, proj(1)x4, proj(2)x4]  (+ urgent d0-1/d2
            #            exchanges; agrb2 pops before the d0+d1 AllGather)
            fills0 = []
            for d in range(1, ND):
                fills0.append(lambda d=d: emit_k(d, 0, sce=True))
                fills0.append(lambda d=d: emit_q(d, 0, sce=True))
            for d in range(ND):
                fills0.append(lambda d=d: emit_q(d, 1, sce=True))
            for d in range(ND):
                fills0.append(lambda d=d: emit_k(d, 1, sce=True))
            for t in range(4, 8):
                fills0.append(lambda t=t: emit_v(t, sce=True))
            attention_chunk(0, fills0)

            dinv0, norms0, agrb0 = epi_pieces(0)
            fills1 = [lambda d=d: emit_q(d, 2, sce=True) for d in range(ND)]
            fills1.append(dinv0)
            fills1 += [lambda d=d: emit_k(d, 2) for d in range(ND)]
            fills1 += norms0
            fills1 += [lambda t=t: emit_v(t) for t in range(10, 12)]
            fills1.append(agrb0)
            attention_chunk(1, fills1)

            dinv1, norms1, agrb1 = epi_pieces(1)
            fills2 = [lambda t=t: emit_v(t) for t in range(8, 10)]
            fills2 += [lambda d=d: emit_q(d, 3) for d in range(ND)]
            fills2.append(dinv1)
            fills2 += norms1
            fills2.append(agrb1)
            fills2 += [lambda et=et: op_proj(0, et) for et in range(ETM)]
            attention_chunk(2, fills2)

            dinv2, norms2, agrb2 = epi_pieces(2)
            fills3 = [lambda: emit_k(0, 3), lambda: emit_v(12)]
            fills3 += [lambda: emit_k(1, 3), lambda: emit_v(13)]
            fills3 += [lambda: emit_v(14), lambda: emit_v(15)]
            fills3 += [lambda: emit_k(2, 3), lambda: emit_k(3, 3)]
            fills3.append(dinv2)
            fills3 += norms2
            fills3.append(agrb2)
            fills3 += [lambda et=et: op_proj(1, et) for et in range(ETM)]
            fills3 += [lambda et=et: op_proj(2, et) for et in range(ETM)]
            attention_chunk(3, fills3)
            # ---- tail: overlap the final AllGather with the projection ----
            while urgent:
                urgent.pop(0)[1]()
            warm = patt.tile([P, SC], F32, tag="att", name="warm")

            def warmers(n):
                # dead matmuls keep HAM at full clock across waits
                for _ in range(n):
                    nc.tensor.matmul(
                        warm[:, 0:P], wq_sb[:, 0, 0:P], xs[0][0][:, 0:P],
                        start=True, stop=True,
                    )

            def proj_tail(et, acc, kds, stop=False, bias=False):
                for kd in kds:
                    nc.tensor.matmul(
                        acc[:],
                        wo_sb[:, kd, P * et : P * (et + 1)],
                        ag_tiles[last][kd][:],
                        start=(kd == 0),
                        stop=(stop and kd == kds[-1]),
                    )
                if bias:
                    stage = outp.tile([P, SC], BF16, tag="ot")
                    nc.vector.tensor_scalar_add(
                        stage[:], acc[:], bo_sb[:, et : et + 1]
                    )
                    nc.sync.dma_start(
                        outT[last, P * et : P * (et + 1), :], stage[:]
                    )

            # wave 1 phase A: kd {0,1,4,5} for et 0/1 are resident since the
            # mid-chunk exchange; they run the moment the attention drains,
            # then warmers bridge the AllGather+readback window
            acc0 = pmisc.tile([P, SC], F32, tag="m", name="acc3_0")
            acc1 = pmisc.tile([P, SC], F32, tag="m", name="acc3_1")
            proj_tail(0, acc0, [0, 1, 4, 5])
            proj_tail(1, acc1, [0, 1, 4, 5])
            warmers(80)
            # AG3b-gated readbacks, split across both hwdge queues;
            # denominators first (they gate the longest chain)
            dinv_b4 = dnp.tile([1, 4 * SC], BF16, tag="dnrb", name="dinv_b4", bufs=1)
            nc.scalar.dma_start(dinv_b4[0:1, :], cc_ag3b[:, 2 * P : 2 * P + 2, :])
            a3 = []
            for r in range(2):
                a2 = agp.tile([P, SC], BF16, tag="ag", name=f"ag2_{r}")
                nc.sync.dma_start(a2[:], cc_ag3b[r, 0:P, :])
                ag_tiles[last][ND * r + 2] = a2
                a = agp.tile([P, SC], BF16, tag="ag", name=f"ag3_{r}")
                nc.scalar.dma_start(a[:], cc_ag3b[r, P : 2 * P, :])
                ag_tiles[last][ND * r + 3] = a
                a3.append(a)
            # wave 1 phase B: d2's blocks at readback, then normalize both
            # ranks' raw d3 tiles, finish kd {3,7}, bias, store
            proj_tail(0, acc0, [2, 6])
            proj_tail(1, acc1, [2, 6])
            bc3 = patt.tile([P, SC], F32, tag="att", name="bc3")
            for r in range(2):
                for h in range(2):
                    c0 = SC * (2 * r + h)
                    nc.tensor.matmul(
                        bc3[:],
                        selh_sb[0:1, P * h : P * (h + 1)],
                        dinv_b4[0:1, c0 : c0 + SC],
                        start=(h == 0),
                        stop=(h == 1),
                    )
                nc.vector.tensor_mul(a3[r][:], a3[r][:], bc3[:])
            proj_tail(0, acc0, [3, 7], stop=True, bias=True)
            proj_tail(1, acc1, [3, 7], stop=True, bias=True)
            # wave 2: et 2/3 re-use the freed accumulators with all eight kd
            # blocks resident
            acc2 = pmisc.tile([P, SC], F32, tag="m", name="acc3_2")
            acc3 = pmisc.tile([P, SC], F32, tag="m", name="acc3_3")
            proj_tail(2, acc2, [0, 1, 4, 5, 2, 6, 3, 7], stop=True, bias=True)
            proj_tail(3, acc3, [0, 1, 4, 5, 2, 6, 3, 7], stop=True, bias=True)

    nc.compile()
    return nc


def _get_runner():
    """Build (once) and return a callable in_maps -> list of out_maps."""
    if "runner" in _CACHE:
        return _CACHE["runner"]

    nc = _build_nc()
    _CACHE["nc"] = nc

    import jax
    from jax.sharding import Mesh, PartitionSpec
    from jax.experimental.shard_map import shard_map
    from concourse import bass2jax
    from concourse.bass2jax import _bass_exec_p, partition_id_tensor

    bass2jax.install_neuronx_cc_hook()

    in_names, out_names, out_avals, zero_shapes = [], [], [], []
    partition_name = nc.partition_id_tensor.name if nc.partition_id_tensor else None
    for alloc in nc.m.functions[0].allocations:
        if not isinstance(alloc, mybir.MemoryLocationSet):
            continue
        name = alloc.memorylocations[0].name
        if alloc.kind == "ExternalInput":
            if name != partition_name:
                in_names.append(name)
        elif alloc.kind == "ExternalOutput":
            out_names.append(name)
            shape = tuple(alloc.tensor_shape)
            dtype = mybir.dt.np(alloc.dtype)
            out_avals.append(jax.core.ShapedArray(shape, dtype))
            zero_shapes.append((shape, dtype))
    n_params = len(in_names)
    all_in_names = list(in_names) + list(out_names)
    if partition_name is not None:
        all_in_names.append(partition_name)

    def _body(*args):
        operands = list(args)
        if partition_name is not None:
            operands.append(partition_id_tensor())
        outs = _bass_exec_p.bind(
            *operands,
            out_avals=tuple(out_avals),
            in_names=tuple(all_in_names),
            out_names=tuple(out_names),
            lowering_input_output_aliases=(),
            sim_require_finite=True,
            sim_require_nnan=True,
            nc=nc,
        )
        return tuple(outs)

    devices = jax.devices()[:NCORES]
    mesh = Mesh(np.asarray(devices), ("core",))
    n_outs = len(out_names)
    sharded = jax.jit(
        shard_map(
            _body,
            mesh=mesh,
            in_specs=(PartitionSpec("core"),) * (n_params + n_outs),
            out_specs=(PartitionSpec("core"),) * n_outs,
            check_rep=False,
        ),
        donate_argnums=tuple(range(n_params, n_params + n_outs)),
        keep_unused=True,
    )

    def runner(in_maps):
        per_core = [[np.asarray(m[name]) for name in in_names] for m in in_maps]
        concat_in = [
            np.concatenate([per_core[c][i] for c in range(NCORES)], axis=0)
            for i in range(n_params)
        ]
        concat_zeros = [
            np.zeros((NCORES * s[0], *s[1:]), d) for (s, d) in zero_shapes
        ]
        out_arrs = sharded(*concat_in, *concat_zeros)
        return [
            {
                name: np.asarray(out_arrs[i]).reshape(NCORES, *out_avals[i].shape)[c]
                for i, name in enumerate(out_names)
            }
            for c in range(NCORES)
        ]

    _CACHE["runner"] = runner
    _CACHE["sharded"] = sharded
    _CACHE["mesh"] = mesh
    _CACHE["meta"] = (in_names, out_names, zero_shapes)
    return runner


def timing_setup(in_maps):
    """Device-resident timing: returns (make_zeros, call).

    `call(make_zeros())` runs one on-device execution with inputs already
    resident (zeros are donated output buffers, created outside the timer).
    """
    _get_runner()
    import jax
    from jax.sharding import NamedSharding, PartitionSpec

    in_names, out_names, zero_shapes = _CACHE["meta"]
    sharding = NamedSharding(_CACHE["mesh"], PartitionSpec("core"))
    per_core = [[np.asarray(m[name]) for name in in_names] for m in in_maps]
    dev_in = [
        jax.device_put(
            np.concatenate([per_core[c][i] for c in range(NCORES)], axis=0), sharding
        )
        for i in range(len(in_names))
    ]
    jax.block_until_ready(dev_in)

    def make_zeros():
        zs = [
            jax.device_put(np.zeros((NCORES * s[0], *s[1:]), d), sharding)
            for (s, d) in zero_shapes
        ]
        jax.block_until_ready(zs)
        return zs

    def call(zs):
        out = _CACHE["sharded"](*dev_in, *zs)
        jax.block_until_ready(out)
        return out

    return make_zeros, call


def make_in_maps(x, Wq, Wk, Wv, Wo, bo):
    """Host-side sharding: slice/transpose/cast full inputs into per-core maps."""
    x = np.asarray(x, dtype=np.float32)
    Wq = np.asarray(Wq, dtype=np.float32)
    Wk = np.asarray(Wk, dtype=np.float32)
    Wv = np.asarray(Wv, dtype=np.float32)
    Wo = np.asarray(Wo, dtype=np.float32)
    bo = np.asarray(bo, dtype=np.float32)
    bf = ml_dtypes.bfloat16

    mask = np.triu(np.ones((P, P), dtype=bf))  # keep t <= s
    sel8 = np.zeros((HL, ND * P), dtype=bf)
    for d in range(ND):
        sel8[2 * d, P * d : P * d + D] = 1
        sel8[2 * d + 1, P * d + D : P * (d + 1)] = 1
    sel = np.zeros((2, P), dtype=bf)
    sel[0, 0:D] = 1
    sel[1, D:P] = 1
    selh = np.zeros((1, 2 * P), dtype=bf)
    selh[0, 0:D] = 1
    selh[0, P + D : 2 * P] = 1
    WoT = np.ascontiguousarray(Wo.T)  # [dg_full, e]
    in_maps = []
    for c in range(NCORES):
        b, g = c // 2, c % 2
        xT = np.ascontiguousarray(x[b].T).astype(bf)  # [E, S]
        wq = np.ascontiguousarray(
            Wq[HL * g : HL * (g + 1)].transpose(1, 0, 2).reshape(E, DG)
        ).astype(bf)
        wk = np.ascontiguousarray(
            Wk[HL * g : HL * (g + 1)].transpose(1, 0, 2).reshape(E, DG)
        ).astype(bf)
        wv = np.ascontiguousarray(
            Wv[HL * g : HL * (g + 1)].transpose(1, 0, 2).reshape(E, DG)
        ).astype(bf)
        wo2 = np.ascontiguousarray(WoT[:, EH * g : EH * (g + 1)]).astype(bf)
        bo2 = np.ascontiguousarray(
            bo[EH * g : EH * (g + 1)].reshape(ETM, P).T
        )  # [P, ETM]
        in_maps.append(
            {
                "xT": xT,
                "wq": wq,
                "wk": wk,
                "wv": wv,
                "wo2": wo2,
                "bo2": bo2,
                "mask": mask,
                "sel8": sel8,
                "sel2": sel,
                "selh": selh,
            }
        )
    return in_maps


def assemble_output(results):
    """Gather per-core outT [EH, S] slices into the full [B, S, E] output."""
    out = np.empty((B, S, E), dtype=np.float32)
    for c in range(NCORES):
        b, g = c // 2, c % 2
        o = results[c]["outT"]  # [NSC, EH, SC]
        out[b, :, EH * g : EH * (g + 1)] = (
            o.transpose(0, 2, 1).reshape(S, EH).astype(np.float32)
        )
    return out


def kernel(x, Wq, Wk, Wv, Wo, bo):
    runner = _get_runner()
    in_maps = make_in_maps(x, Wq, Wk, Wv, Wo, bo)
    results = runner(in_maps)
    return assemble_output(results)


# revision 61
# speedup vs baseline: 1.0156x; 1.0156x over previous
"""Causal multi-head attention on 8 Trainium2 NeuronCores.

Problem: x[B=4,S=2048,E=1024], Wq/Wk/Wv[H=16,E,D=64], Wo[E,E], bo[E].
  out = softmax_causal(q k^T / sqrt(D)) v, heads concat, @ Wo.T + bo

Sharding (tensor parallel over heads, data parallel over batch):
  core c -> (batch b = c//2, head-group g = c%2 of 8 heads).
  Each core: QKV projections + attention for its 8 heads of its batch.
  Output exchange: instead of ReduceScatter-ing full-width output
  partials (v3), each core AllGathers its NORMALIZED attention tiles
  (one [128, 512] block per head-pair d) with its pair peer, then
  computes the output projection for ONLY its own 512 output columns
  with the full 1024-dim contraction (Wo columns sliced per core, full
  rows).  Same PE work, no partial-sum adds, collective payloads are
  small and pipeline with compute.  Collectives cost ~10-20us each
  regardless of size (latency-bound), so there are exactly 5: one per
  chunk (popped as fills one chunk later, a full chunk before that
  chunk's projection needs the data), and the last chunk splits into an
  early d0+d1 exchange (fires ~50% through the chunk, always done
  before the tail) and a final one carrying d2-normalized + d3-RAW +
  d3's two denominator rows.  Shipping d3 raw moves the whole
  reciprocal/broadcast/normalize chain off the pre-AllGather critical
  path: both sides normalize the gathered d3 blocks identically while
  the projection's first 16 matmuls (kd 0,1,4,5, resident since the
  early exchange) and a string of dead warm-keeper matmuls (HAM stays
  at 2.4 GHz) bridge the AllGather latency; kd 2,6 join at readback and
  kd 3,7 land last.

Kernel internals (per core):
  - All SBUF data bf16 (psum f32).  Activations transposed: xT[E,S],
    QT/KT[dg,S], scoresT[t,q]; softmax denominator from a ones-column
    appended to V; probabilities feed the AV matmul moving operand.
  - Scores for a head PAIR (row-tiled 64-contraction matmuls at PE row
    groups 0/64) share one [128, 1024] psum tile; ONE exp activation
    covers both heads.
  - Causality at 128 granularity: suffix-restricted scores/exp/AV on
    diagonal tiles + a [128,128] upper-tri wedge multiply.
  - Denominator rows are extracted right after each AV accumulation
    (vector queue only, never stalls PE); each chunk's reciprocal/
    broadcast/normalize runs as fills in the NEXT chunk, with the dinv
    fill placed several iterations before the first norm so the PE
    broadcast matmul never waits on the reciprocal chain.
  - Phase-1 (QKV) and out-projection work interleaved into the attention
    emission so TensorE always has dense work while ScalarE exp runs;
    late K/V emission is shifted into the ACT-bound last chunk.
"""

import os
import sys

for _p in ("/opt/trn_rl_repo", "/root/.axon_site/_ro/trn_rl_repo"):
    if os.path.isdir(_p) and _p not in sys.path:
        sys.path.append(_p)

import numpy as np
import ml_dtypes

import concourse.bass as bass
import concourse.mybir as mybir
import concourse.tile as tile
from concourse import bacc

B, S, E, H, D = 4, 2048, 1024, 16, 64
NCORES = 8
G = 2  # head groups
HL = H // G  # heads per core = 8
DG = HL * D  # local head dim = 512
EH = E // G  # final output columns per core = 512
KD = 2 * (DG // 128)  # global 128-dim blocks of the full head dim = 8
ETM = EH // 128  # my output col blocks = 4
P = 128
SC = 512  # sequence chunk
NSC = S // SC  # 4
NT = S // P  # 16 key tiles
ET = E // P  # 8 embedding tiles
ND = DG // P  # 4 head-pairs per core
SCALE = 1.0 / np.sqrt(D)
PAIRS = [[0, 1], [2, 3], [4, 5], [6, 7]]

F32 = mybir.dt.float32
BF16 = mybir.dt.bfloat16

_CACHE = {}


def _build_nc():
    nc = bacc.Bacc("TRN2", target_bir_lowering=False, debug=False, num_devices=NCORES)

    xT = nc.dram_tensor("xT", [E, S], BF16, kind="ExternalInput")
    wq = nc.dram_tensor("wq", [E, DG], BF16, kind="ExternalInput")
    wk = nc.dram_tensor("wk", [E, DG], BF16, kind="ExternalInput")
    wv = nc.dram_tensor("wv", [E, DG], BF16, kind="ExternalInput")
    wo2 = nc.dram_tensor("wo2", [2 * DG, EH], BF16, kind="ExternalInput")
    bo2 = nc.dram_tensor("bo2", [P, ETM], F32, kind="ExternalInput")
    mask = nc.dram_tensor("mask", [P, P], BF16, kind="ExternalInput")
    sel8 = nc.dram_tensor("sel8", [HL, ND * P], BF16, kind="ExternalInput")
    sel2 = nc.dram_tensor("sel2", [2, P], BF16, kind="ExternalInput")
    selh = nc.dram_tensor("selh", [1, 2 * P], BF16, kind="ExternalInput")
    outT = nc.dram_tensor("outT", [NSC, EH, SC], BF16, kind="ExternalOutput")

    with tile.TileContext(nc) as tc:
        with (
            tc.tile_pool(name="persist", bufs=1) as persist,
            tc.tile_pool(name="expp", bufs=5) as expp,
            tc.tile_pool(name="attnp", bufs=8) as attnp,
            tc.tile_pool(name="agp", bufs=16) as agp,
            tc.tile_pool(name="dnp", bufs=4) as dnp,
            tc.tile_pool(name="dinvp", bufs=3) as dinvp,
            tc.tile_pool(name="workp", bufs=4) as workp,
            tc.tile_pool(name="outp", bufs=4) as outp,
            tc.tile_pool(name="psc", bufs=2, space="PSUM") as psc,
            tc.tile_pool(name="patt", bufs=2, space="PSUM") as patt,
            tc.tile_pool(name="pmisc", bufs=2, space="PSUM") as pmisc,
            tc.tile_pool(name="dram", bufs=1, space="DRAM") as dram,
        ):
            # ---- persistent tiles ----
            xs = [
                [persist.tile([P, SC], BF16, name=f"x{e}_{c}") for c in range(NSC)]
                for e in range(ET)
            ]
            wq_sb = persist.tile([P, ET, DG], BF16, name="wq")
            wk_sb = persist.tile([P, ET, DG], BF16, name="wk")
            wv_sb = persist.tile([P, ET, DG], BF16, name="wv")
            wo_sb = persist.tile([P, KD, EH], BF16, name="wo")
            bo_sb = persist.tile([P, ETM], F32, name="bo")
            mask_sb = persist.tile([P, P], BF16, name="mask")
            sel8_sb = persist.tile([HL, ND * P], BF16, name="sel8")
            sel2_sb = persist.tile([2, P], BF16, name="sel2")
            selh_sb = persist.tile([1, 2 * P], BF16, name="selh")
            kt = [
                [persist.tile([P, SC], BF16, name=f"kt{d}_{kc}") for kc in range(NSC)]
                for d in range(ND)
            ]
            qt = [
                [persist.tile([P, SC], BF16, name=f"qt{d}_{sc}") for sc in range(NSC)]
                for d in range(ND)
            ]
            v_sb = [persist.tile([P, HL, D + 1], BF16, name=f"v{t}") for t in range(NT)]

            cc_at = dram.tile([NSC, ND, P, SC], BF16)
            cc_ag = [dram.tile([2, ND, P, SC], BF16, name=f"ccag{c}") for c in range(3)]
            cc_ag3a = dram.tile([2, 2, P, SC], BF16)
            # final exchange: d2 normalized + d3 RAW + 2 denominator rows
            cc_at3b = dram.tile([2 * P + 2, SC], BF16)
            cc_ag3b = dram.tile([2, 2 * P + 2, SC], BF16)
            # throwaway payload: a dummy AllGather right before the final one
            # absorbs the cold-start cost of the collective stream
            cc_warm = dram.tile([1, SC], BF16)
            cc_warmg = dram.tile([2, SC], BF16)

            # ---- input DMAs: per-e weight blocks interleaved with x slices
            # so K(0)'s e-th matmul can start as soon as its operands land.
            # The first-needed blocks go on the Scalar (ACT) hwdge queue,
            # whose engine preamble retires earlier than Sync's.
            # tiny constants on the otherwise-idle scalar queue: the wedge
            # mask gates every diagonal AV of chunk 0
            nc.scalar.dma_start(mask_sb[:], mask[:])
            nc.scalar.dma_start(sel8_sb[:], sel8[:])
            nc.scalar.dma_start(sel2_sb[:], sel2[:])
            nc.scalar.dma_start(selh_sb[:], selh[:])
            nc.scalar.dma_start(bo_sb[:], bo2[:])
            for e in range(ET):
                nc.sync.dma_start(
                    wk_sb[:, e, :], wk[P * e : P * (e + 1), :]
                )
                nc.sync.dma_start(xs[e][0][:], xT[P * e : P * (e + 1), 0:SC])
            for e in range(ET):
                nc.sync.dma_start(wq_sb[:, e, :], wq[P * e : P * (e + 1), :])
            for e in range(ET):
                nc.sync.dma_start(wv_sb[:, e, :], wv[P * e : P * (e + 1), :])
            for c in range(1, NSC):
                for e in range(ET):
                    nc.sync.dma_start(
                        xs[e][c][:], xT[P * e : P * (e + 1), SC * c : SC * (c + 1)]
                    )
            nc.sync.dma_start(wo_sb[:], wo2.rearrange("(ko p) m -> p ko m", p=P))
            for t in range(NT):
                nc.vector.memset(v_sb[t][:, :, D], 1.0)
            # zero the two score-psum ring slots once: merged diagonal exps
            # read a junk gap between the two halves, which must be finite
            for _ in range(2):
                sco0 = psc.tile([P, 2 * SC], F32, tag="sc", name="sco_init")
                nc.vector.memset(sco0[:], 0.0)

            # ---- phase-1 emitters ----
            def emit_k(d, kc, w_sb=wk_sb, dst=kt, sce=False):
                acc = pmisc.tile([P, SC], F32, tag="m", name="acc")
                for e in range(ET):
                    nc.tensor.matmul(
                        acc[:],
                        w_sb[:, e, P * d : P * (d + 1)],
                        xs[e][kc][:],
                        start=(e == 0),
                        stop=(e == ET - 1),
                    )
                # sce: free the psum slot via the (idle) scalar engine so the
                # next fill's matmul never waits on a backed-up DVE queue
                (nc.scalar.copy if sce else nc.vector.tensor_copy)(
                    dst[d][kc][:], acc[:]
                )

            def emit_q(d, sc, sce=False):
                emit_k(d, sc, w_sb=wq_sb, dst=qt, sce=sce)

            def emit_v(t, sce=False):
                acc = pmisc.tile([P, DG], F32, tag="m", name="accv")
                for e in range(ET):
                    nc.tensor.matmul(
                        acc[:],
                        xs[e][t // 4][:, P * (t % 4) : P * (t % 4 + 1)],
                        wv_sb[:, e, :],
                        start=(e == 0),
                        stop=(e == ET - 1),
                    )
                (nc.scalar.copy if sce else nc.vector.tensor_copy)(
                    v_sb[t][:, :, 0:D], acc[:].rearrange("p (h d) -> p h d", d=D)
                )

            # ---- per-chunk epilogue: normalize, AllGather, project ----
            # kd order is GLOBAL: rank0's d0..3 -> kd 0..3, rank1's -> 4..7,
            # matching wo2's row order (full head-dim, global head index).
            ag_tiles = [[None] * KD for _ in range(NSC)]
            at_tiles = [[None] * ND for _ in range(NSC)]
            dn_locs = [None] * NSC  # [HL, SC] f32, denominators of a chunk

            def op_dinv(dn_ap, rows, dinv_b, sce=False):
                dinv_f = dinvp.tile([rows, SC], F32, tag="dif", name="dinv_f")
                nc.vector.reciprocal_approx_fast(dinv_f[:], dn_ap)
                (nc.scalar.copy if sce else nc.vector.tensor_copy)(
                    dinv_b[:], dinv_f[:]
                )

            def op_norm(sc, d, sel_ap, dinv_b, sce=False):
                # broadcast 1/den across each head's 64 rows, normalize the
                # raw AV tile in place, stage it for the AllGather
                bc_ps = pmisc.tile([P, SC], F32, tag="m", name="bc_ps")
                nc.tensor.matmul(bc_ps[:], sel_ap, dinv_b[:], start=True, stop=True)
                bc = workp.tile([P, SC], BF16, tag="bc")
                (nc.scalar.copy if sce else nc.vector.tensor_copy)(bc[:], bc_ps[:])
                at_d = at_tiles[sc][d]
                nc.vector.tensor_mul(at_d[:], at_d[:], bc[:])
                nc.sync.dma_start(cc_at[sc, d], at_d[:])

            def op_ag(in_ap, out_ap):
                nc.gpsimd.collective_compute(
                    "AllGather",
                    mybir.AluOpType.bypass,
                    replica_groups=PAIRS,
                    ins=[in_ap.opt()],
                    outs=[out_ap.opt()],
                )

            def op_readback(sc, out_tile, ds):
                # pull both ranks' gathered blocks back into SBUF
                for r in range(2):
                    for j, d in enumerate(ds):
                        a = agp.tile([P, SC], BF16, tag="ag", name=f"ag{sc}_{r}_{d}")
                        nc.sync.dma_start(a[:], out_tile[r, j])
                        ag_tiles[sc][ND * r + d] = a

            # last-chunk accumulation order: the blocks fed by the final
            # AllGather (kd 3 and 7) go last so the rest overlaps it
            KD_ORDER = [0, 1, 2, 4, 5, 6, 3, 7]

            def op_proj(sc, et):
                acc = pmisc.tile([P, SC], F32, tag="m", name="acco")
                for i, kd in enumerate(KD_ORDER):
                    nc.tensor.matmul(
                        acc[:],
                        wo_sb[:, kd, P * et : P * (et + 1)],
                        ag_tiles[sc][kd][:],
                        start=(i == 0),
                        stop=(i == KD - 1),
                    )
                stage = outp.tile([P, SC], BF16, tag="ot")
                nc.vector.tensor_scalar_add(stage[:], acc[:], bo_sb[:, et : et + 1])
                nc.sync.dma_start(outT[sc, P * et : P * (et + 1), :], stage[:])

            def epi_pieces(sc):
                # epilogue pieces for chunks 0..2, interleaved as fills in
                # chunk sc+1: (dinv, [norm x4], ag+readback)
                dinv_b = dinvp.tile([HL, SC], BF16, tag="dib", name="dinv_b")
                f_dinv = lambda: op_dinv(dn_locs[sc][:], HL, dinv_b)
                f_norms = [
                    (
                        lambda d=d: op_norm(
                            sc, d, sel8_sb[:, P * d : P * (d + 1)], dinv_b
                        )
                    )
                    for d in range(ND)
                ]

                def f_agrb():
                    op_ag(cc_at[sc], cc_ag[sc])
                    op_readback(sc, cc_ag[sc], range(ND))

                return f_dinv, f_norms, f_agrb

            # ---- attention ----
            # `urgent` holds the last chunk's own mid-chunk epilogue
            # closures: each entry is (ready_git, fn) and pops only a few
            # iterations after creation so its PE matmul never waits on the
            # freshly-issued reciprocal chain.
            urgent = []
            git = [0]  # global iteration counter across chunks
            last = NSC - 1
            dn2p = [None]  # last chunk d2's denominators, partition-0 based

            def attention_chunk(sc, fills):
                nt = 4 * (sc + 1)
                n_iter = ND * nt
                it = 0
                popped = 0
                dn_locs[sc] = dnp.tile([HL, SC], F32, tag="dnl", name=f"dn{sc}")

                for d in range(ND):
                    att = [
                        patt.tile([D + 1, SC], F32, tag="att", name=f"att{h}")
                        for h in range(2)
                    ]

                    def emit_av(j, o, ex, d=d, att=att, nt=nt):
                        for half in range(2):
                            nc.tensor.matmul(
                                att[half][:, o:SC],
                                v_sb[j][:, 2 * d + half, :],
                                ex[:, SC * half + o : SC * (half + 1)],
                                start=(j == 0),
                                stop=(j == nt - 1),
                            )

                    prev = None
                    for j in range(nt):
                        m = j - 4 * sc
                        o = P * m if m > 0 else 0
                        sco = psc.tile([P, 2 * SC], F32, tag="sc", name="sco")
                        for half in range(2):
                            r = D * half
                            nc.tensor.matmul(
                                sco[:, SC * half + o : SC * (half + 1)],
                                kt[d][j // 4][r : r + D, P * (j % 4) : P * (j % 4 + 1)],
                                qt[d][sc][r : r + D, o:SC],
                                start=True,
                                stop=True,
                                tile_position=(r, 0),
                            )
                        ex = expp.tile([P, 2 * SC], BF16, tag="ex", name="ex")
                        # one exp covers both halves; for diagonal tiles it
                        # spans the unwritten gap between them (finite junk,
                        # never read downstream) to save the per-op overhead
                        nc.scalar.activation(
                            ex[:, o : 2 * SC],
                            sco[:, o : 2 * SC],
                            mybir.ActivationFunctionType.Exp,
                            scale=SCALE,
                        )
                        if m >= 0:  # diagonal tile: mask the wedge
                            for half in range(2):
                                c0 = SC * half + P * m
                                nc.vector.tensor_mul(
                                    ex[:, c0 : c0 + P], ex[:, c0 : c0 + P], mask_sb[:]
                                )
                        if prev is not None:
                            emit_av(*prev)
                        prev = (j, o, ex)
                        it += 1
                        git[0] += 1
                        if urgent and git[0] >= urgent[0][0]:
                            urgent.pop(0)[1]()
                        want = (it * len(fills)) // n_iter + 1
                        while popped < min(want, len(fills)):
                            fills[popped]()
                            popped += 1
                    emit_av(*prev)
                    # stage raw AV + extract denominators (vector/DMA only;
                    # nothing here blocks the PE queue)
                    attn_t = attnp.tile([P, SC], BF16, tag="at", name="attn_t")
                    at_tiles[sc][d] = attn_t
                    if sc == last and d == ND - 1:
                        # final head-pair: ship the UNNORMALIZED AV plus its
                        # two denominator rows (packed on partition 0); both
                        # sides normalize after the exchange, off the pre-AG
                        # critical path
                        dn3p = dnp.tile([1, 2 * SC], F32, tag="dn3", name="dn3p", bufs=1)
                        for half in range(2):
                            nc.scalar.copy(
                                dn3p[0:1, SC * half : SC * (half + 1)],
                                att[half][D : D + 1, :],
                            )
                            nc.vector.tensor_copy(
                                attn_t[D * half : D * (half + 1), :],
                                att[half][0:D, :],
                            )
                        # reciprocal BEFORE the exchange: both sides receive
                        # ready-to-use 1/den rows, so the post-AG chain is
                        # just broadcast-multiply-project
                        dinv_f3 = dinvp.tile(
                            [1, 2 * SC], F32, tag="dif3", name="dinv_f3", bufs=1
                        )
                        nc.vector.reciprocal_approx_fast(dinv_f3[:], dn3p[0:1, :])
                        dinv3b = dnp.tile(
                            [1, 2 * SC], BF16, tag="dn3b", name="dinv3b", bufs=1
                        )
                        nc.scalar.copy(dinv3b[:], dinv_f3[:])
                        nc.scalar.dma_start(cc_at3b[P : 2 * P, :], attn_t[:])
                        nc.scalar.dma_start(
                            cc_at3b[2 * P : 2 * P + 2, :], dinv3b[0:1, :]
                        )
                        op_ag(cc_at3b, cc_ag3b)
                        continue
                    cp = nc.scalar.copy if sc == 0 else nc.vector.tensor_copy
                    for half in range(2):
                        cp(
                            attn_t[D * half : D * (half + 1), :], att[half][0:D, :]
                        )
                        dnrow = dnp.tile([1, SC], F32, tag="dn", name="dnrow", bufs=2)
                        cp(dnrow[:], att[half][D : D + 1, :])
                        if sc == last and d == 2:
                            if dn2p[0] is None:
                                dn2p[0] = dnp.tile(
                                    [2, SC], F32, tag="dn2p", name="dn2p", bufs=1
                                )
                            nc.sync.dma_start(
                                dn2p[0][half : half + 1, :], dnrow[:]
                            )
                        else:
                            row = 2 * d + half
                            nc.sync.dma_start(
                                dn_locs[sc][row : row + 1, :], dnrow[:]
                            )
                    if sc == last and d == 1:
                        # the last chunk's first two head-pairs normalize and
                        # AllGather mid-chunk (always done before the tail);
                        # delayed pops keep PE off the recip chain
                        dinv4 = dinvp.tile([4, SC], BF16, tag="dib", name="dinv4")
                        urgent.extend(
                            [
                                (
                                    git[0] + 3,
                                    lambda: op_dinv(dn_locs[sc][0:4, :], 4, dinv4),
                                ),
                            ]
                            + [
                                (
                                    git[0] + 4 + dd,
                                    lambda dd=dd: op_norm(
                                        sc,
                                        dd,
                                        sel8_sb[0:4, P * dd : P * (dd + 1)],
                                        dinv4,
                                    ),
                                )
                                for dd in range(2)
                            ]
                            + [
                                (
                                    git[0] + 9,
                                    lambda: op_ag(cc_at[sc, 0:2], cc_ag3a),
                                ),
                                (
                                    git[0] + 10,
                                    lambda: op_readback(sc, cc_ag3a, range(2)),
                                ),
                            ]
                        )
                    if sc == last and d == 2:
                        # d2 normalizes mid-chunk and stages into the final
                        # exchange buffer (its dn rows live on partitions 0-1)
                        dinv2p = dinvp.tile([2, SC], BF16, tag="dib", name="dinv2p")
                        urgent.extend(
                            [
                                (git[0] + 3, lambda: op_dinv(dn2p[0][:], 2, dinv2p)),
                                (
                                    git[0] + 4,
                                    lambda: op_norm(sc, 2, sel2_sb[:], dinv2p),
                                ),
                                (
                                    git[0] + 5,
                                    lambda: nc.sync.dma_start(
                                        cc_at3b[0:P, :], at_tiles[sc][2][:]
                                    ),
                                ),
                                # the dummy AG must not be hoisted: feed it
                                # from a tile that only exists late so it
                                # runs right before the final exchange
                                (
                                    git[0] + 6,
                                    lambda: nc.sync.dma_start(
                                        cc_warm[0:1, :], dinv2p[0:1, :]
                                    ),
                                ),
                                (git[0] + 8, lambda: op_ag(cc_warm, cc_warmg)),
                            ]
                        )
                while popped < len(fills):
                    fills[popped]()
                    popped += 1

            # ---- schedule ----
            # minimal prologue: only what chunk-0 d0's attention needs
            emit_k(0, 0, sce=True)
            emit_q(0, 0, sce=True)
            for t in range(4):
                emit_v(t, sce=True)

            # fill layout per chunk (positions paced over the chunk, +1
            # prefetch).  Emitters sit in the LAST chunk that still meets
            # their first-use iteration, pushing PE work into the exp-bound
            # tail chunks:
            #   chunk0: [K/Q d1-3, Q(1)x4, K(1)x4, V4-7]
            #   chunk1: [Q(2)x4, dinv(0), K(2)x4, norm(0)x4, V10, V11, agrb0]
            #   chunk2: [V8, V9, Q(3)x4, dinv(1), norm(1)x4, agrb1, proj(0)x4]
            #   chunk3: [K30, V14, K31, V15, K32, K33, dinv(2), norm(2)x4,
            #            agrb2, proj(1)x4, proj(2)x4]  (+ urgent d0-1/d2
            #            exchanges; agrb2 pops before the d0+d1 AllGather)
            fills0 = []
            for d in range(1, ND):
                fills0.append(lambda d=d: emit_k(d, 0, sce=True))
                fills0.append(lambda d=d: emit_q(d, 0, sce=True))
            for d in range(ND):
                fills0.append(lambda d=d: emit_q(d, 1, sce=True))
            for d in range(ND):
                fills0.append(lambda d=d: emit_k(d, 1, sce=True))
            for t in range(4, 8):
                fills0.append(lambda t=t: emit_v(t, sce=True))
            attention_chunk(0, fills0)

            dinv0, norms0, agrb0 = epi_pieces(0)
            fills1 = [lambda d=d: emit_q(d, 2, sce=True) for d in range(ND)]
            fills1.append(dinv0)
            fills1 += [lambda d=d: emit_k(d, 2) for d in range(ND)]
            fills1 += norms0
            fills1 += [lambda t=t: emit_v(t) for t in range(10, 12)]
            fills1.append(agrb0)
            attention_chunk(1, fills1)

            dinv1, norms1, agrb1 = epi_pieces(1)
            fills2 = [lambda t=t: emit_v(t) for t in range(8, 10)]
            fills2 += [lambda d=d: emit_q(d, 3) for d in range(ND)]
            fills2.append(dinv1)
            fills2 += norms1
            fills2.append(agrb1)
            fills2 += [lambda et=et: op_proj(0, et) for et in range(ETM)]
            attention_chunk(2, fills2)

            dinv2, norms2, agrb2 = epi_pieces(2)
            fills3 = [lambda: emit_k(0, 3), lambda: emit_v(12)]
            fills3 += [lambda: emit_k(1, 3), lambda: emit_v(13)]
            fills3 += [lambda: emit_v(14), lambda: emit_v(15)]
            fills3 += [lambda: emit_k(2, 3), lambda: emit_k(3, 3)]
            fills3.append(dinv2)
            fills3 += norms2
            fills3.append(agrb2)
            fills3 += [lambda et=et: op_proj(1, et) for et in range(ETM)]
            fills3 += [lambda et=et: op_proj(2, et) for et in range(ETM)]
            attention_chunk(3, fills3)
            # ---- tail: overlap the final AllGather with the projection ----
            while urgent:
                urgent.pop(0)[1]()
            warm = patt.tile([P, SC], F32, tag="att", name="warm")

            def warmers(n, pin=None):
                # dead matmuls keep HAM at full clock across waits.  With
                # pin=, only the FIRST one reads the late-written tile (the
                # rest chain behind it via the psum WAW), so the scheduler
                # cannot hoist the batch out of the window it must bridge
                # and no per-op semaphore cost is added
                for i in range(n):
                    mov = pin if (pin is not None and i == 0) else xs[0][0]
                    nc.tensor.matmul(
                        warm[:, 0:P], wq_sb[:, 0, 0:P], mov[:, 0:P],
                        start=True, stop=True,
                    )

            def proj_tail(et, acc, kds, stop=False, bias=False):
                for kd in kds:
                    nc.tensor.matmul(
                        acc[:],
                        wo_sb[:, kd, P * et : P * (et + 1)],
                        ag_tiles[last][kd][:],
                        start=(kd == 0),
                        stop=(stop and kd == kds[-1]),
                    )
                if bias:
                    stage = outp.tile([P, SC], BF16, tag="ot")
                    nc.vector.tensor_scalar_add(
                        stage[:], acc[:], bo_sb[:, et : et + 1]
                    )
                    nc.sync.dma_start(
                        outT[last, P * et : P * (et + 1), :], stage[:]
                    )

            # wave 1 phase A: kd {0,1,4,5} for et 0/1 are resident since the
            # mid-chunk exchange; they run the moment the attention drains,
            # then warmers bridge the AllGather+readback window
            acc0 = pmisc.tile([P, SC], F32, tag="m", name="acc3_0")
            acc1 = pmisc.tile([P, SC], F32, tag="m", name="acc3_1")
            proj_tail(0, acc0, [0, 1, 4, 5])
            proj_tail(1, acc1, [0, 1, 4, 5])
            warmers(80, pin=at_tiles[last][ND - 1])
            # AG3b-gated readbacks, split across both hwdge queues;
            # denominators first (they gate the longest chain)
            dinv_b4 = dnp.tile([1, 4 * SC], BF16, tag="dnrb", name="dinv_b4", bufs=1)
            nc.scalar.dma_start(dinv_b4[0:1, :], cc_ag3b[:, 2 * P : 2 * P + 2, :])
            a3 = []
            for r in range(2):
                a2 = agp.tile([P, SC], BF16, tag="ag", name=f"ag2_{r}")
                nc.sync.dma_start(a2[:], cc_ag3b[r, 0:P, :])
                ag_tiles[last][ND * r + 2] = a2
                a = agp.tile([P, SC], BF16, tag="ag", name=f"ag3_{r}")
                nc.scalar.dma_start(a[:], cc_ag3b[r, P : 2 * P, :])
                ag_tiles[last][ND * r + 3] = a
                a3.append(a)
            # wave 1 phase B: d2's blocks at readback, then normalize both
            # ranks' raw d3 tiles, finish kd {3,7}, bias, store
            proj_tail(0, acc0, [2, 6])
            proj_tail(1, acc1, [2, 6])
            bc3 = patt.tile([P, SC], F32, tag="att", name="bc3")
            for r in range(2):
                for h in range(2):
                    c0 = SC * (2 * r + h)
                    nc.tensor.matmul(
                        bc3[:],
                        selh_sb[0:1, P * h : P * (h + 1)],
                        dinv_b4[0:1, c0 : c0 + SC],
                        start=(h == 0),
                        stop=(h == 1),
                    )
                nc.vector.tensor_mul(a3[r][:], a3[r][:], bc3[:])
            proj_tail(0, acc0, [3, 7], stop=True, bias=True)
            proj_tail(1, acc1, [3, 7], stop=True, bias=True)
            # wave 2: et 2/3 re-use the freed accumulators with all eight kd
            # blocks resident
            acc2 = pmisc.tile([P, SC], F32, tag="m", name="acc3_2")
            acc3 = pmisc.tile([P, SC], F32, tag="m", name="acc3_3")
            proj_tail(2, acc2, [0, 1, 4, 5, 2, 6, 3, 7], stop=True, bias=True)
            proj_tail(3, acc3, [0, 1, 4, 5, 2, 6, 3, 7], stop=True, bias=True)

    nc.compile()
    return nc


def _get_runner():
    """Build (once) and return a callable in_maps -> list of out_maps."""
    if "runner" in _CACHE:
        return _CACHE["runner"]

    nc = _build_nc()
    _CACHE["nc"] = nc

    import jax
    from jax.sharding import Mesh, PartitionSpec
    from jax.experimental.shard_map import shard_map
    from concourse import bass2jax
    from concourse.bass2jax import _bass_exec_p, partition_id_tensor

    bass2jax.install_neuronx_cc_hook()

    in_names, out_names, out_avals, zero_shapes = [], [], [], []
    partition_name = nc.partition_id_tensor.name if nc.partition_id_tensor else None
    for alloc in nc.m.functions[0].allocations:
        if not isinstance(alloc, mybir.MemoryLocationSet):
            continue
        name = alloc.memorylocations[0].name
        if alloc.kind == "ExternalInput":
            if name != partition_name:
                in_names.append(name)
        elif alloc.kind == "ExternalOutput":
            out_names.append(name)
            shape = tuple(alloc.tensor_shape)
            dtype = mybir.dt.np(alloc.dtype)
            out_avals.append(jax.core.ShapedArray(shape, dtype))
            zero_shapes.append((shape, dtype))
    n_params = len(in_names)
    all_in_names = list(in_names) + list(out_names)
    if partition_name is not None:
        all_in_names.append(partition_name)

    def _body(*args):
        operands = list(args)
        if partition_name is not None:
            operands.append(partition_id_tensor())
        outs = _bass_exec_p.bind(
            *operands,
            out_avals=tuple(out_avals),
            in_names=tuple(all_in_names),
            out_names=tuple(out_names),
            lowering_input_output_aliases=(),
            sim_require_finite=True,
            sim_require_nnan=True,
            nc=nc,
        )
        return tuple(outs)

    devices = jax.devices()[:NCORES]
    mesh = Mesh(np.asarray(devices), ("core",))
    n_outs = len(out_names)
    sharded = jax.jit(
        shard_map(
            _body,
            mesh=mesh,
            in_specs=(PartitionSpec("core"),) * (n_params + n_outs),
            out_specs=(PartitionSpec("core"),) * n_outs,
            check_rep=False,
        ),
        donate_argnums=tuple(range(n_params, n_params + n_outs)),
        keep_unused=True,
    )

    def runner(in_maps):
        per_core = [[np.asarray(m[name]) for name in in_names] for m in in_maps]
        concat_in = [
            np.concatenate([per_core[c][i] for c in range(NCORES)], axis=0)
            for i in range(n_params)
        ]
        concat_zeros = [
            np.zeros((NCORES * s[0], *s[1:]), d) for (s, d) in zero_shapes
        ]
        out_arrs = sharded(*concat_in, *concat_zeros)
        return [
            {
                name: np.asarray(out_arrs[i]).reshape(NCORES, *out_avals[i].shape)[c]
                for i, name in enumerate(out_names)
            }
            for c in range(NCORES)
        ]

    _CACHE["runner"] = runner
    _CACHE["sharded"] = sharded
    _CACHE["mesh"] = mesh
    _CACHE["meta"] = (in_names, out_names, zero_shapes)
    return runner


def timing_setup(in_maps):
    """Device-resident timing: returns (make_zeros, call).

    `call(make_zeros())` runs one on-device execution with inputs already
    resident (zeros are donated output buffers, created outside the timer).
    """
    _get_runner()
    import jax
    from jax.sharding import NamedSharding, PartitionSpec

    in_names, out_names, zero_shapes = _CACHE["meta"]
    sharding = NamedSharding(_CACHE["mesh"], PartitionSpec("core"))
    per_core = [[np.asarray(m[name]) for name in in_names] for m in in_maps]
    dev_in = [
        jax.device_put(
            np.concatenate([per_core[c][i] for c in range(NCORES)], axis=0), sharding
        )
        for i in range(len(in_names))
    ]
    jax.block_until_ready(dev_in)

    def make_zeros():
        zs = [
            jax.device_put(np.zeros((NCORES * s[0], *s[1:]), d), sharding)
            for (s, d) in zero_shapes
        ]
        jax.block_until_ready(zs)
        return zs

    def call(zs):
        out = _CACHE["sharded"](*dev_in, *zs)
        jax.block_until_ready(out)
        return out

    return make_zeros, call


def make_in_maps(x, Wq, Wk, Wv, Wo, bo):
    """Host-side sharding: slice/transpose/cast full inputs into per-core maps."""
    x = np.asarray(x, dtype=np.float32)
    Wq = np.asarray(Wq, dtype=np.float32)
    Wk = np.asarray(Wk, dtype=np.float32)
    Wv = np.asarray(Wv, dtype=np.float32)
    Wo = np.asarray(Wo, dtype=np.float32)
    bo = np.asarray(bo, dtype=np.float32)
    bf = ml_dtypes.bfloat16

    mask = np.triu(np.ones((P, P), dtype=bf))  # keep t <= s
    sel8 = np.zeros((HL, ND * P), dtype=bf)
    for d in range(ND):
        sel8[2 * d, P * d : P * d + D] = 1
        sel8[2 * d + 1, P * d + D : P * (d + 1)] = 1
    sel = np.zeros((2, P), dtype=bf)
    sel[0, 0:D] = 1
    sel[1, D:P] = 1
    selh = np.zeros((1, 2 * P), dtype=bf)
    selh[0, 0:D] = 1
    selh[0, P + D : 2 * P] = 1
    WoT = np.ascontiguousarray(Wo.T)  # [dg_full, e]
    in_maps = []
    for c in range(NCORES):
        b, g = c // 2, c % 2
        xT = np.ascontiguousarray(x[b].T).astype(bf)  # [E, S]
        wq = np.ascontiguousarray(
            Wq[HL * g : HL * (g + 1)].transpose(1, 0, 2).reshape(E, DG)
        ).astype(bf)
        wk = np.ascontiguousarray(
            Wk[HL * g : HL * (g + 1)].transpose(1, 0, 2).reshape(E, DG)
        ).astype(bf)
        wv = np.ascontiguousarray(
            Wv[HL * g : HL * (g + 1)].transpose(1, 0, 2).reshape(E, DG)
        ).astype(bf)
        wo2 = np.ascontiguousarray(WoT[:, EH * g : EH * (g + 1)]).astype(bf)
        bo2 = np.ascontiguousarray(
            bo[EH * g : EH * (g + 1)].reshape(ETM, P).T
        )  # [P, ETM]
        in_maps.append(
            {
                "xT": xT,
                "wq": wq,
                "wk": wk,
                "wv": wv,
                "wo2": wo2,
                "bo2": bo2,
                "mask": mask,
                "sel8": sel8,
                "sel2": sel,
                "selh": selh,
            }
        )
    return in_maps


def assemble_output(results):
    """Gather per-core outT [EH, S] slices into the full [B, S, E] output."""
    out = np.empty((B, S, E), dtype=np.float32)
    for c in range(NCORES):
        b, g = c // 2, c % 2
        o = results[c]["outT"]  # [NSC, EH, SC]
        out[b, :, EH * g : EH * (g + 1)] = (
            o.transpose(0, 2, 1).reshape(S, EH).astype(np.float32)
        )
    return out


def kernel(x, Wq, Wk, Wv, Wo, bo):
    runner = _get_runner()
    in_maps = make_in_maps(x, Wq, Wk, Wv, Wo, bo)
    results = runner(in_maps)
    return assemble_output(results)
